# revision 45
# baseline (speedup 1.0000x reference)
"""Trainium2 Bass kernel for Swin-style windowed attention w/ relative position bias.

Problem: x[8, 1025, 768], 12 heads, head_dim 64, rel-pos bias table gathered
by a constant index matrix. Sharding: pure data-parallel - one batch element
per NeuronCore (8 cores).

v2 dataflow (all matmuls bf16; fp32 PSUM accumulation):
  xT [768, 1028]          x[b].T, bf16
  qkT = wqk.T @ xT        -> [1536, 1028] (q/k per head pair, q pre-scaled)
  v   = xT.T @ wv         -> [1025, 768]  (tokens-major, ones col for rowsum)
  S.T[k,q] = k_h @ q_h.T  (K=64, PSUM tile [128,1024] spanning 2 banks,
                           q tail col 1024 in a shared tail bank)
  P.T = exp(S.T) * expB   (one wide ACT exp evacuates PSUM -> bf16; bias is
                           applied multiplicatively: expB = exp(bias) is
                           precomputed on host and streamed bf16, so the DVE
                           multiply runs on all-bf16 SBUF operands)
  [O.T; rowsum] = [v_h|1].T @ P.T
  O.T *= rcp(rowsum)      (DVE fast reciprocal + gpsimd partition broadcast;
                           no Ln/table switches on ACT - one act table total)
  outT = wproj.T @ O_all + projb_eff,  projb_eff = proj_b + proj_w @ v_bias
                           (v_bias folded exactly into the proj bias on host)
"""

import sys

import numpy as np

for _p in ("/opt/trn_rl_repo",):
    if _p not in sys.path:
        sys.path.insert(0, _p)

B = 8
N = 1025
NP = 1028        # q padded: 2 full PSUM banks + 4-wide tail
C = 768
H = 12
D = 64
SCALE = D ** -0.5
NKT = 9          # k tiles of 128 (8 full + 1)
QCHUNKS = [(0, 512), (512, 512)]          # main q chunks (tail separate)
QCHUNKS3 = [(0, 512), (512, 512), (1024, 4)]
NCHUNKS_V = [(0, 512), (512, 256)]
LAG = 4          # S->PV software pipeline depth (units); deeper for head 0
LAG0 = 6         # (absorbs the phase1->phase2 engine-queue debt)
LAGB = 3         # expB DMA prefetch depth (units)
DEBUG = False    # set by debug harness; adds intermediate dump outputs


def _emit(ctx, tc, xT, wqk, wqkb, wv, expb, expb_tail, wproj, projb, outT,
          dbg=None):
    import concourse.mybir as mybir

    nc = tc.nc
    f32 = mybir.dt.float32
    bf16 = mybir.dt.bfloat16
    AF = mybir.ActivationFunctionType

    def ktsize(kt):
        return 128 if kt < 8 else 1

    lp = nc.allow_low_precision(
        reason="bf16 matmul inputs / elementwise; f32 PSUM accumulation")
    lp.__enter__()
    ctx.callback(lambda: lp.__exit__(None, None, None))

    # Long-lived pools first (pool release must be LIFO / stack ordered).
    # The PE HAM clock gate only un-throttles (1.2 -> 2.4 GHz) for matmuls
    # whose stationary spans all 128 K rows; K=64 matmuls anywhere in the
    # stream keep the core cold. So every matmul here is K=128: the S
    # stationaries are per-head zero-padded k tiles (kz), and v_sb[8]'s
    # invalid rows are zeroed so the kt=8 PV can contract over 128 rows.
    qk_pool = ctx.enter_context(tc.tile_pool(name="qk_pool", bufs=1))
    v_pool = ctx.enter_context(tc.tile_pool(name="v_pool", bufs=1))
    q_sb = []
    for pair in range(6):
        t = qk_pool.tile([128, NP], bf16, tag=f"q{pair}", name=f"q{pair}")
        q_sb.append(t)
    kz_sb = []
    for h in range(12):
        t = qk_pool.tile([128, NP], bf16, tag=f"kz{h}", name=f"kz{h}")
        kz_sb.append(t)
        # zero the opposite head's 64 rows (once; never overwritten)
        nc.gpsimd.memset(t[(1 - h % 2) * 64:(2 - h % 2) * 64, :], 0.0)
    v_sb = []
    for kt in range(NKT):
        t = v_pool.tile([128, H, D + 1], bf16, tag=f"v{kt}", name=f"v{kt}")
        v_sb.append(t)
        if kt < 8:
            nc.vector.memset(t[:, :, D:D + 1], 1.0)
        else:
            # only row 0 is a real token; zero the rest so K=128 PV is exact
            nc.vector.memset(t[:, :, :], 0.0)
            nc.vector.memset(t[0:1, :, D:D + 1], 1.0)

    opool = ctx.enter_context(tc.tile_pool(name="opool", bufs=1))
    o_all = []
    for m in range(6):
        t = opool.tile([128, NP], bf16, tag=f"oall{m}", name=f"oall{m}")
        o_all.append(t)

    wp_pool = ctx.enter_context(tc.tile_pool(name="wp_pool", bufs=1))
    wproj_t = []
    projb_t = []
    for ct in range(6):
        t = wp_pool.tile([128, 768], bf16, tag=f"wproj{ct}", name=f"wproj{ct}")
        wproj_t.append(t)
        tb = wp_pool.tile([128, 1], f32, tag=f"projb{ct}", name=f"projb{ct}")
        projb_t.append(tb)

    # expB stream pools (addresses must not overlap phase-1 tiles so the
    # DMAs can prefetch during the QKV projections).
    bpool = ctx.enter_context(tc.tile_pool(name="bpool", bufs=LAGB + 2))
    btail_pool = ctx.enter_context(tc.tile_pool(name="btail", bufs=2))

    # ---------------- phase 1: load weights + x, QKV projections -------------
    xpool = tc.alloc_tile_pool(name="xpool", bufs=1)
    wvpool = tc.alloc_tile_pool(name="wvpool", bufs=1)
    wqkpool = tc.alloc_tile_pool(name="wqkpool", bufs=1)

    # DMA issue order matters: x + wv first (v matmuls unblock soonest),
    # wqk after (only needed ~20us in), wproj/projb last (phase 3).
    wqk_t, wv_t, xT_t = [], [], []
    for ct in range(6):
        xt = xpool.tile([128, NP], bf16, tag=f"xT{ct}", name=f"xT{ct}")
        nc.sync.dma_start(xt[:, :], xT[ct * 128:(ct + 1) * 128, :])
        xT_t.append(xt)
        w2 = wvpool.tile([128, 768], bf16, tag=f"wv{ct}", name=f"wv{ct}")
        nc.sync.dma_start(w2[:, :], wv[ct * 128:(ct + 1) * 128, :])
        wv_t.append(w2)
        w1 = wqkpool.tile([128, 1536], bf16, tag=f"wqk{ct}", name=f"wqk{ct}")
        wqk_t.append(w1)
    wqkb_t = wqkpool.tile([128, 12], f32, tag="wqkb", name="wqkb")
    for ct in range(6):
        nc.sync.dma_start(wqk_t[ct][:, :], wqk[ct * 128:(ct + 1) * 128, :])
    nc.sync.dma_start(wqkb_t[:, :], wqkb[:, :])
    for ct in range(6):
        nc.sync.dma_start(wproj_t[ct][:, :], wproj[ct * 128:(ct + 1) * 128, :])
        nc.sync.dma_start(projb_t[ct][:, :], projb[ct * 128:(ct + 1) * 128, :])

    with tc.tile_pool(name="ps1", bufs=4, space="PSUM") as ps1, \
         tc.tile_pool(name="ps1v", bufs=2, space="PSUM") as ps1v:
        # v first (needs only xT+wv): PE starts sooner. ct-outer/chunk-inner
        # so consecutive matmuls share a stationary (ldweights dedupe).
        for kt in range(NKT):
            p = ktsize(kt)
            ps = ps1v.tile([128, 768], f32, tag="ps1v", name=f"ps1v_{kt}")
            for ct in range(6):
                for (c0, cn) in NCHUNKS_V:
                    nc.tensor.matmul(
                        ps[:p, c0:c0 + cn],
                        xT_t[ct][:, kt * 128: kt * 128 + p],
                        wv_t[ct][:, c0:c0 + cn],
                        start=(ct == 0), stop=(ct == 5),
                    )
            nc.scalar.copy(
                v_sb[kt][:p, :, 0:D],
                ps[:p, :].rearrange("p (h d) -> p h d", h=H),
            )
        # q/k: qkT[m-tile, n] = sum_ct wqk[ct, m-tile].T @ xT[ct, n]
        # q m-tiles (even) evacuate whole; k m-tiles (odd) split per head
        # into the zero-padded kz stationaries.
        for m in range(12):
            pair = m // 2
            pss = [ps1.tile([128, 512], f32, tag="ps1t", name=f"ps1_{m}_{q0}")
                   for (q0, qn) in QCHUNKS3]
            for ct in range(6):
                for ci, (q0, qn) in enumerate(QCHUNKS3):
                    nc.tensor.matmul(
                        pss[ci][:, :qn],
                        wqk_t[ct][:, m * 128:(m + 1) * 128],
                        xT_t[ct][:, q0:q0 + qn],
                        start=(ct == 0), stop=(ct == 5),
                    )
            for ci, (q0, qn) in enumerate(QCHUNKS3):
                ps = pss[ci]
                if m % 2 == 0:
                    nc.scalar.activation(q_sb[pair][:, q0:q0 + qn], ps[:, :qn],
                                         AF.Identity, bias=wqkb_t[:, m:m + 1])
                else:
                    nc.scalar.copy(kz_sb[2 * pair][0:64, q0:q0 + qn],
                                   ps[0:64, :qn])
                    nc.scalar.copy(kz_sb[2 * pair + 1][64:128, q0:q0 + qn],
                                   ps[64:128, :qn])

    wqkpool.release()
    wvpool.release()
    xpool.release()

    if dbg is not None:
        for pair in range(6):
            nc.sync.dma_start(dbg["qk"][pair], q_sb[pair][:, :])
        for h in range(6):
            nc.sync.dma_start(dbg["qk"][6 + h], kz_sb[h][:, :])
        for kt in range(NKT):
            nc.sync.dma_start(
                dbg["v"][kt], v_sb[kt][:, :, :].rearrange("p h d -> p (h d)"))

    # ---------------- phase 2: attention, one head at a time -----------------
    ppool = ctx.enter_context(tc.tile_pool(name="ppool", bufs=LAG0 + 2))
    p0pool = ctx.enter_context(tc.tile_pool(name="p0pool", bufs=2))
    ptpool = ctx.enter_context(tc.tile_pool(name="ptpool", bufs=2))
    npool = ctx.enter_context(tc.tile_pool(name="npool", bufs=2))
    ocpool = ctx.enter_context(tc.tile_pool(name="ocpool", bufs=2))

    bts = {}

    def issue_bdma(u):
        if u >= 12 * NKT:
            return
        h, kt = divmod(u, NKT)
        p = ktsize(kt)
        bt = bpool.tile([128, 1024], bf16, tag="bt", name=f"bt_{h}_{kt}")
        nc.sync.dma_start(bt[:p, :], expb[h, kt, 0:p, :])
        bts[u] = bt

    for u in range(LAGB):
        issue_bdma(u)

    # PSUM budget (8 banks): S tiles 2x[128,1024] = 4, o_ps 2x[65,512] = 2,
    # tail tiles 2x[128,48] = 2.
    def make_norm_stage(pair, pr0, ocp, rsm, q0, qn, h, ci):
        # reciprocal + broadcast + scale, reading the SBUF evacuation of
        # o_ps; deferred into the NEXT head's units so the DVE queue always
        # serves the next head's P multiplies first. rsm must be a
        # partition-0 SBUF tile (custom-DVE reciprocal and the gpsimd
        # partition broadcast both only read physical partition 0).
        def stage():
            rs = npool.tile([1, 512], f32, tag="rs", name=f"rs_{h}_{ci}")
            nc.vector.reciprocal_approx_fast(rs[:, :qn], rsm[:, :qn])
            bc = npool.tile([64, 512], f32, tag="bc", name=f"bc_{h}_{ci}")
            nc.gpsimd.partition_broadcast(bc[:, :qn], rs[:, :qn])
            nc.vector.tensor_mul(
                o_all[pair][pr0:pr0 + 64, q0:q0 + qn],
                ocp[0:64, :qn], bc[:, :qn])
        return stage

    with tc.tile_pool(name="ps_s", bufs=2, space="PSUM") as ps_s, \
         tc.tile_pool(name="ps_o", bufs=1, space="PSUM") as ps_o, \
         tc.tile_pool(name="ps_t", bufs=2, space="PSUM") as ps_t:

        pending = []
        for h in range(12):
            pair, hh = divmod(h, 2)
            pr0 = hh * 64
            q_t = q_sb[pair]
            k_t = kz_sb[h]
            lag = LAG0 if h == 0 else LAG

            o_ps = [
                ps_o.tile([65, 512], f32, tag="o0", name=f"o0_{h}"),
                ps_o.tile([65, 512], f32, tag="o1", name=f"o1_{h}"),
            ]
            # shared tail bank: S tail cols [0:36], O tail accum [0:65, 40:44]
            st = ps_t.tile([128, 48], f32, tag="st", name=f"st_{h}")

            btl = btail_pool.tile([128, 36], bf16, tag="btl", name=f"btl_{h}")
            nc.sync.dma_start(btl[:, :], expb_tail[h, :, :])

            pes = {}

            def pv_unit(kt):
                # K=128 always: v_sb[8] rows past the real token are zeroed,
                # so stale (finite) pe rows contribute nothing.
                pe = pes.pop(kt)
                for ci, (q0, qn) in enumerate(QCHUNKS):
                    nc.tensor.matmul(
                        o_ps[ci][:, :],
                        v_sb[kt][:, h, :],
                        pe[:, q0:q0 + qn],
                        start=(kt == 0), stop=(kt == 8),
                    )

            for kt in range(NKT):
                issue_bdma(h * NKT + kt + LAGB)
                p = ktsize(kt)
                s2 = ps_s.tile([128, 1024], f32, tag="s2", name=f"s2_{h}_{kt}")
                for (q0, qn) in QCHUNKS:
                    nc.tensor.matmul(
                        s2[:p, q0:q0 + qn],
                        k_t[:, kt * 128: kt * 128 + p],
                        q_t[:, q0:q0 + qn],
                        start=True, stop=True,
                    )
                nc.tensor.matmul(
                    st[:p, 4 * kt:4 * kt + 4],
                    k_t[:, kt * 128: kt * 128 + p],
                    q_t[:, 1024:1028],
                    start=True, stop=True,
                )
                if kt == 0:
                    nc.vector.memset(st[0:65, 40:44], 0.0)
                pe0 = p0pool.tile([128, 1024], bf16, tag="pe0",
                                  name=f"pe0_{h}_{kt}")
                nc.scalar.activation(pe0[:p, :], s2[:p, :], AF.Exp)
                pe = ppool.tile([128, 1024], bf16, tag="pe",
                                name=f"pe_{h}_{kt}")
                nc.vector.tensor_mul(pe[:p, :], pe0[:p, :],
                                     bts[h * NKT + kt][:p, :])
                pes[kt] = pe
                if dbg is not None and h == 0:
                    nc.sync.dma_start(dbg["pe"][kt, 0:p, :], pe[:p, :])
                if pending:
                    pending.pop(0)()
                if kt >= lag:
                    pv_unit(kt - lag)
            for kt in range(NKT - lag, NKT):
                pv_unit(kt)

            # evacuate O + rowsum to SBUF immediately - this frees the o_ps
            # PSUM banks so the next head's PV can start; normalize math is
            # deferred into the next head's units. rowsum rows go to
            # partition-0 tiles (custom-DVE reciprocal requirement), with
            # the copies split across DVE and ACT for balance.
            ocp0 = ocpool.tile([65, 512], f32, tag="ocp0", name=f"ocp0_{h}")
            nc.vector.tensor_copy(ocp0[:, :], o_ps[0][:, :])
            rsm0 = ocpool.tile([1, 512], f32, tag="rsm0", name=f"rsm0_{h}")
            nc.vector.tensor_copy(rsm0[:, :], o_ps[0][64:65, :])
            ocp1 = ocpool.tile([65, 512], f32, tag="ocp1", name=f"ocp1_{h}")
            nc.vector.tensor_copy(ocp1[:, :], o_ps[1][:, :])
            rsm1 = ocpool.tile([1, 512], f32, tag="rsm1", name=f"rsm1_{h}")
            nc.vector.tensor_copy(rsm1[:, :], o_ps[1][64:65, :])

            # q-tail (token 1024): one exp+mult for all 9 k tiles, then the
            # 9 tiny PV matmuls accumulate into the shared tail bank region.
            pt0 = ptpool.tile([128, 36], bf16, tag="pt0", name=f"pt0_{h}")
            nc.scalar.activation(pt0[:, :], st[:, 0:36], AF.Exp)
            ptl = ptpool.tile([128, 36], bf16, tag="ptl", name=f"ptl_{h}")
            nc.vector.tensor_mul(ptl[:, :], pt0[:, :], btl[:, :])
            for kt in range(NKT):
                nc.tensor.matmul(
                    st[0:65, 40:44],
                    v_sb[kt][:, h, :],
                    ptl[:, 4 * kt:4 * kt + 4],
                    start=False, stop=(kt == 8),
                    skip_group_check=True,
                )
            ocpt = ocpool.tile([65, 4], f32, tag="ocpt", name=f"ocpt_{h}")
            nc.vector.tensor_copy(ocpt[:, :], st[0:65, 40:44])
            rsmt = ocpool.tile([1, 4], f32, tag="rsmt", name=f"rsmt_{h}")
            nc.vector.tensor_copy(rsmt[:, :], st[64:65, 40:44])
            if dbg is not None and h == 0:
                nc.sync.dma_start(dbg["ops"][0], ocp0[:, :])
                nc.sync.dma_start(dbg["ops"][1], ocp1[:, :])

            pending.append(make_norm_stage(pair, pr0, ocp0, rsm0, 0, 512, h, 0))
            pending.append(make_norm_stage(pair, pr0, ocp1, rsm1, 512, 512, h, 1))
            pending.append(make_norm_stage(pair, pr0, ocpt, rsmt, 1024, 4, h, 2))
        for s in pending:
            s()

    if dbg is not None:
        for m in range(6):
            nc.sync.dma_start(dbg["oall"][m], o_all[m][:, :])

    # ---------------- phase 3: output projection ----------------
    # ct-outer/chunk-inner so consecutive matmuls share a wproj stationary.
    with tc.tile_pool(name="ops3", bufs=2, space="PSUM") as ps3, \
         tc.tile_pool(name="out_pool", bufs=3) as out_pool:
        for m in range(6):
            pss = [ps3.tile([128, 512], f32, tag=f"ps3_{ci}",
                            name=f"ps3_{m}_{ci}")
                   for ci in range(3)]
            for ct in range(6):
                for ci, (q0, qn) in enumerate(QCHUNKS3):
                    nc.tensor.matmul(
                        pss[ci][:, :qn],
                        wproj_t[ct][:, m * 128:(m + 1) * 128],
                        o_all[ct][:, q0:q0 + qn],
                        start=(ct == 0), stop=(ct == 5),
                    )
            for ci, (q0, qn) in enumerate(QCHUNKS3):
                wn = min(qn, N - q0)
                ot = out_pool.tile([128, 512], f32, tag="ot",
                                   name=f"ot_{m}_{q0}")
                nc.scalar.activation(ot[:, :wn], pss[ci][:, :wn], AF.Identity,
                                     bias=projb_t[m])
                nc.sync.dma_start(outT[m * 128:(m + 1) * 128, q0:q0 + wn],
                                  ot[:, :wn])


def _host_prep(x, qkv_w, q_bias, v_bias, rpb_table, proj_w, proj_b,
               rel_pos_index):
    """Layout transforms + exact bias folds; bf16 casts for matmul inputs."""
    import ml_dtypes
    f = np.float32
    bf = ml_dtypes.bfloat16
    x = np.asarray(x, f)
    qkv_w = np.asarray(qkv_w, f)
    q_bias = np.asarray(q_bias, f)
    v_bias = np.asarray(v_bias, f)
    rpb_table = np.asarray(rpb_table, f)
    proj_w = np.asarray(proj_w, f)
    proj_b = np.asarray(proj_b, f)
    idx = np.asarray(rel_pos_index)

    # q/k weights: column blocks [q0 q1 | k0 k1 | q2 q3 | k2 k3 | ...],
    # q pre-scaled by 1/sqrt(D); q/k biases added at PSUM evacuation.
    rows = []
    brows = []
    for p in range(6):
        rows.append(qkv_w[p * 128:(p + 1) * 128] * SCALE)
        brows.append(q_bias[p * 128:(p + 1) * 128] * SCALE)
        rows.append(qkv_w[C + p * 128: C + (p + 1) * 128])
        brows.append(np.zeros(128, f))
    wqk = np.concatenate(rows, axis=0)               # [1536, 768]
    wqk_bias = np.concatenate(brows, axis=0)         # [1536]
    wqk_aug = np.ascontiguousarray(wqk.T).astype(bf)  # [768, 1536]
    wqkb = np.ascontiguousarray(wqk_bias.reshape(12, 128).T)  # [128, 12] f32

    wv = np.ascontiguousarray(qkv_w[2 * C:3 * C].T).astype(bf)  # [768, 768]

    rpb = rpb_table[idx]                              # [N, N, H] (q, k, h)
    biasT = np.ascontiguousarray(rpb.transpose(2, 1, 0))  # [H, k, q]
    expb_pad = np.zeros((H, NKT * 128, NP), f)
    expb_pad[:, :N, :N] = biasT
    expb_pad = np.exp(expb_pad)                       # pads exp(0) = 1
    expb = np.ascontiguousarray(
        expb_pad[:, :, :1024].reshape(H, NKT, 128, 1024)).astype(bf)
    # tail: q cols 1024:1028 for each k tile, packed [H, 128, 4*NKT]
    tail = expb_pad[:, :, 1024:1028].reshape(H, NKT, 128, 4)
    expb_tail = np.ascontiguousarray(
        tail.transpose(0, 2, 1, 3).reshape(H, 128, 4 * NKT)).astype(bf)

    wproj = np.ascontiguousarray(proj_w.T).astype(bf)  # [768, 768]
    # v_bias folded exactly into the projection bias:
    # out += v_bias @ proj_w.T == proj_w @ v_bias (per output channel)
    projb_eff = proj_b + proj_w @ v_bias
    projb = np.ascontiguousarray(projb_eff.reshape(C, 1)).astype(f)

    xT = np.zeros((B, C, NP), bf)
    for b in range(B):
        xT[b, :, :N] = x[b].T.astype(bf)
    return xT, wqk_aug, wqkb, wv, expb, expb_tail, wproj, projb


def _dedupe_ldweights(nc):
    """Remove InstLdweights that reload the identical stationary AP as the
    immediately preceding PE ldweights (only matmuls in between, which leave
    the loaded weights intact). The duplicate's dependants are remapped to
    the surviving ldweights; incoming deps are merged (identical anyway
    since the stationary AP is identical)."""
    import concourse.mybir as mybir
    PE = mybir.EngineType.PE
    n_del = 0
    for blk in nc.main_func.blocks:
        keep = []
        last_key = None
        last_ldw = None
        for inst in blk.instructions:
            tn = type(inst).__name__
            if getattr(inst, "engine", None) == PE:
                if tn == "InstLdweights":
                    key = str(inst.ins[0])
                    si = inst.sync_info
                    clean = si is None or (
                        len(si.on_wait) == 0 and len(si.on_update) == 0)
                    if key == last_key and clean and last_ldw is not None:
                        for dname in list(inst.descendants or []):
                            dep = nc.inst_map.get(dname)
                            if dep is not None:
                                try:
                                    dep.remap_dependency_names(
                                        {inst.name: last_ldw.name})
                                except Exception:
                                    pass
                        try:
                            last_ldw.merge_dependencies_from(inst)
                        except Exception:
                            pass
                        n_del += 1
                        continue
                    last_key = key
                    last_ldw = inst
                elif tn != "InstMatmult":
                    last_key = None
                    last_ldw = None
            keep.append(inst)
        blk.instructions[:] = keep
    return n_del


_BUILT = {}


def _build():
    if "nc" in _BUILT:
        return _BUILT["nc"]
    from contextlib import ExitStack

    import concourse.mybir as mybir
    import concourse.tile as tile
    from concourse import bacc

    nc = bacc.Bacc("TRN2", target_bir_lowering=False, debug=False,
                   num_devices=B)
    f32 = mybir.dt.float32
    bf16 = mybir.dt.bfloat16
    xT = nc.dram_tensor("xT", (C, NP), bf16, kind="ExternalInput").ap()
    wqk = nc.dram_tensor("wqk", (768, 1536), bf16, kind="ExternalInput").ap()
    wqkb = nc.dram_tensor("wqkb", (128, 12), f32, kind="ExternalInput").ap()
    wv = nc.dram_tensor("wv", (768, 768), bf16, kind="ExternalInput").ap()
    expb = nc.dram_tensor("expb", (H, NKT, 128, 1024), bf16,
                          kind="ExternalInput").ap()
    expb_tail = nc.dram_tensor("expb_tail", (H, 128, 4 * NKT), bf16,
                               kind="ExternalInput").ap()
    wproj = nc.dram_tensor("wproj", (768, 768), bf16,
                           kind="ExternalInput").ap()
    projb = nc.dram_tensor("projb", (768, 1), f32, kind="ExternalInput").ap()
    outT = nc.dram_tensor("outT", (768, N), f32, kind="ExternalOutput").ap()

    dbg = None
    if DEBUG:
        dbg = {
            "qk": nc.dram_tensor("dbg_qk", (12, 128, NP), bf16,
                                 kind="ExternalOutput").ap(),
            "v": nc.dram_tensor("dbg_v", (NKT, 128, H * (D + 1)), bf16,
                                kind="ExternalOutput").ap(),
            "pe": nc.dram_tensor("dbg_pe", (NKT, 128, 1024), bf16,
                                 kind="ExternalOutput").ap(),
            "ops": nc.dram_tensor("dbg_ops", (2, 65, 512), f32,
                                  kind="ExternalOutput").ap(),
            "rs": nc.dram_tensor("dbg_rs", (2, 1, 512), f32,
                                 kind="ExternalOutput").ap(),
            "bc": nc.dram_tensor("dbg_bc", (2, 64, 512), f32,
                                 kind="ExternalOutput").ap(),
            "oall": nc.dram_tensor("dbg_oall", (6, 128, NP), bf16,
                                   kind="ExternalOutput").ap(),
        }

    with tile.TileContext(nc) as tc:
        with ExitStack() as ctx:
            _emit(ctx, tc, xT, wqk, wqkb, wv, expb, expb_tail, wproj,
                  projb, outT, dbg=dbg)
    _dedupe_ldweights(nc)
    nc.compile()
    _BUILT["nc"] = nc
    return nc


def kernel(x, qkv_w, q_bias, v_bias, rpb_table, proj_w, proj_b,
           rel_pos_index):
    from concourse.bass_utils import run_bass_kernel_spmd

    xT, wqk, wqkb, wv, expb, expb_tail, wproj, projb = _host_prep(
        x, qkv_w, q_bias, v_bias, rpb_table, proj_w, proj_b, rel_pos_index)

    nc = _build()
    shared = {
        "wqk": wqk, "wqkb": wqkb, "wv": wv, "expb": expb,
        "expb_tail": expb_tail, "wproj": wproj, "projb": projb,
    }
    in_maps = [dict(shared, xT=np.ascontiguousarray(xT[b]))
               for b in range(B)]
    res = run_bass_kernel_spmd(nc, in_maps, core_ids=list(range(B)))
    out = np.stack([res.results[b]["outT"].T for b in range(B)], axis=0)
    return out.astype(np.float32)


# revision 46
# speedup vs baseline: 1.0494x; 1.0494x over previous
"""Trainium2 Bass kernel for Swin-style windowed attention w/ relative position bias.

Problem: x[8, 1025, 768], 12 heads, head_dim 64, rel-pos bias table gathered
by a constant index matrix. Sharding: pure data-parallel - one batch element
per NeuronCore (8 cores).

v2 dataflow (all matmuls bf16; fp32 PSUM accumulation):
  xT [768, 1028]          x[b].T, bf16
  qkT = wqk.T @ xT        -> [1536, 1028] (q/k per head pair, q pre-scaled)
  v   = xT.T @ wv         -> [1025, 768]  (tokens-major, ones col for rowsum)
  S.T[k,q] = k_h @ q_h.T  (K=64, PSUM tile [128,1024] spanning 2 banks,
                           q tail col 1024 in a shared tail bank)
  P.T = exp(S.T) * expB   (one wide ACT exp evacuates PSUM -> bf16; bias is
                           applied multiplicatively: expB = exp(bias) is
                           precomputed on host and streamed bf16, so the DVE
                           multiply runs on all-bf16 SBUF operands)
  [O.T; rowsum] = [v_h|1].T @ P.T
  O.T *= rcp(rowsum)      (DVE fast reciprocal + gpsimd partition broadcast;
                           no Ln/table switches on ACT - one act table total)
  outT = wproj.T @ O_all + projb_eff,  projb_eff = proj_b + proj_w @ v_bias
                           (v_bias folded exactly into the proj bias on host)
"""

import sys

import numpy as np

for _p in ("/opt/trn_rl_repo",):
    if _p not in sys.path:
        sys.path.insert(0, _p)

B = 8
N = 1025
NP = 1028        # q padded: 2 full PSUM banks + 4-wide tail
C = 768
H = 12
D = 64
SCALE = D ** -0.5
NKT = 9          # k tiles of 128 (8 full + 1)
QCHUNKS = [(0, 512), (512, 512)]          # main q chunks (tail separate)
QCHUNKS3 = [(0, 512), (512, 512), (1024, 4)]
NCHUNKS_V = [(0, 512), (512, 256)]
LAG = 4          # S->PV software pipeline depth (units); deeper for head 0
LAG0 = 6         # (absorbs the phase1->phase2 engine-queue debt)
LAGB = 3         # expB DMA prefetch depth (units)
DEBUG = False    # set by debug harness; adds intermediate dump outputs


def _emit(ctx, tc, xT, wqk, wqkb, wv, expb, expb_tail, wproj, projb, outT,
          dbg=None):
    import concourse.mybir as mybir

    nc = tc.nc
    f32 = mybir.dt.float32
    bf16 = mybir.dt.bfloat16
    AF = mybir.ActivationFunctionType

    def ktsize(kt):
        return 128 if kt < 8 else 1

    lp = nc.allow_low_precision(
        reason="bf16 matmul inputs / elementwise; f32 PSUM accumulation")
    lp.__enter__()
    ctx.callback(lambda: lp.__exit__(None, None, None))

    # Long-lived pools first (pool release must be LIFO / stack ordered).
    # The PE HAM clock gate only un-throttles (1.2 -> 2.4 GHz) for matmuls
    # whose stationary spans all 128 K rows; K=64 matmuls anywhere in the
    # stream keep the core cold. So every matmul here is K=128: the S
    # stationaries are per-head zero-padded k tiles (kz), and v_sb[8]'s
    # invalid rows are zeroed so the kt=8 PV can contract over 128 rows.
    qk_pool = ctx.enter_context(tc.tile_pool(name="qk_pool", bufs=1))
    v_pool = ctx.enter_context(tc.tile_pool(name="v_pool", bufs=1))
    q_sb = []
    for pair in range(6):
        t = qk_pool.tile([128, NP], bf16, tag=f"q{pair}", name=f"q{pair}")
        q_sb.append(t)
    kz_sb = []
    for h in range(12):
        t = qk_pool.tile([128, NP], bf16, tag=f"kz{h}", name=f"kz{h}")
        kz_sb.append(t)
        # zero the opposite head's 64 rows (once; never overwritten)
        nc.gpsimd.memset(t[(1 - h % 2) * 64:(2 - h % 2) * 64, :], 0.0)
    v_sb = []
    for kt in range(NKT):
        t = v_pool.tile([128, H, D + 1], bf16, tag=f"v{kt}", name=f"v{kt}")
        v_sb.append(t)
        if kt < 8:
            nc.vector.memset(t[:, :, D:D + 1], 1.0)
        else:
            # only row 0 is a real token; zero the rest so K=128 PV is exact
            nc.vector.memset(t[:, :, :], 0.0)
            nc.vector.memset(t[0:1, :, D:D + 1], 1.0)

    opool = ctx.enter_context(tc.tile_pool(name="opool", bufs=1))
    o_all = []
    for m in range(6):
        t = opool.tile([128, NP], bf16, tag=f"oall{m}", name=f"oall{m}")
        o_all.append(t)

    wp_pool = ctx.enter_context(tc.tile_pool(name="wp_pool", bufs=1))
    wproj_t = []
    projb_t = []
    for ct in range(6):
        t = wp_pool.tile([128, 768], bf16, tag=f"wproj{ct}", name=f"wproj{ct}")
        wproj_t.append(t)
        tb = wp_pool.tile([128, 1], f32, tag=f"projb{ct}", name=f"projb{ct}")
        projb_t.append(tb)

    # expB stream pools (addresses must not overlap phase-1 tiles so the
    # DMAs can prefetch during the QKV projections).
    bpool = ctx.enter_context(tc.tile_pool(name="bpool", bufs=LAGB + 2))
    btail_pool = ctx.enter_context(tc.tile_pool(name="btail", bufs=2))

    # ---------------- phase 1: load weights + x, QKV projections -------------
    xpool = tc.alloc_tile_pool(name="xpool", bufs=1)
    wvpool = tc.alloc_tile_pool(name="wvpool", bufs=1)
    wqkpool = tc.alloc_tile_pool(name="wqkpool", bufs=1)

    # DMA issue order matters: x + wv first (v matmuls unblock soonest),
    # wqk after (only needed ~20us in), wproj/projb last (phase 3).
    wqk_t, wv_t, xT_t = [], [], []
    for ct in range(6):
        xt = xpool.tile([128, NP], bf16, tag=f"xT{ct}", name=f"xT{ct}")
        nc.sync.dma_start(xt[:, :], xT[ct * 128:(ct + 1) * 128, :])
        xT_t.append(xt)
        w2 = wvpool.tile([128, 768], bf16, tag=f"wv{ct}", name=f"wv{ct}")
        nc.sync.dma_start(w2[:, :], wv[ct * 128:(ct + 1) * 128, :])
        wv_t.append(w2)
        w1 = wqkpool.tile([128, 1536], bf16, tag=f"wqk{ct}", name=f"wqk{ct}")
        wqk_t.append(w1)
    wqkb_t = wqkpool.tile([128, 12], f32, tag="wqkb", name="wqkb")
    for ct in range(6):
        nc.sync.dma_start(wqk_t[ct][:, :], wqk[ct * 128:(ct + 1) * 128, :])
    nc.sync.dma_start(wqkb_t[:, :], wqkb[:, :])
    for ct in range(6):
        nc.sync.dma_start(wproj_t[ct][:, :], wproj[ct * 128:(ct + 1) * 128, :])
        nc.sync.dma_start(projb_t[ct][:, :], projb[ct * 128:(ct + 1) * 128, :])

    with tc.tile_pool(name="ps1", bufs=4, space="PSUM") as ps1, \
         tc.tile_pool(name="ps1v", bufs=2, space="PSUM") as ps1v:
        # v first (needs only xT+wv): PE starts sooner. ct-outer/chunk-inner
        # so consecutive matmuls share a stationary (ldweights dedupe).
        for kt in range(NKT):
            p = ktsize(kt)
            ps = ps1v.tile([128, 768], f32, tag="ps1v", name=f"ps1v_{kt}")
            for ct in range(6):
                for (c0, cn) in NCHUNKS_V:
                    nc.tensor.matmul(
                        ps[:p, c0:c0 + cn],
                        xT_t[ct][:, kt * 128: kt * 128 + p],
                        wv_t[ct][:, c0:c0 + cn],
                        start=(ct == 0), stop=(ct == 5),
                    )
            nc.scalar.copy(
                v_sb[kt][:p, :, 0:D],
                ps[:p, :].rearrange("p (h d) -> p h d", h=H),
            )
        # q/k: qkT[m-tile, n] = sum_ct wqk[ct, m-tile].T @ xT[ct, n]
        # q m-tiles (even) evacuate whole; k m-tiles (odd) split per head
        # into the zero-padded kz stationaries.
        for m in range(12):
            pair = m // 2
            pss = [ps1.tile([128, 512], f32, tag="ps1t", name=f"ps1_{m}_{q0}")
                   for (q0, qn) in QCHUNKS3]
            for ct in range(6):
                for ci, (q0, qn) in enumerate(QCHUNKS3):
                    nc.tensor.matmul(
                        pss[ci][:, :qn],
                        wqk_t[ct][:, m * 128:(m + 1) * 128],
                        xT_t[ct][:, q0:q0 + qn],
                        start=(ct == 0), stop=(ct == 5),
                    )
            for ci, (q0, qn) in enumerate(QCHUNKS3):
                ps = pss[ci]
                if m % 2 == 0:
                    nc.scalar.activation(q_sb[pair][:, q0:q0 + qn], ps[:, :qn],
                                         AF.Identity, bias=wqkb_t[:, m:m + 1])
                else:
                    nc.scalar.copy(kz_sb[2 * pair][0:64, q0:q0 + qn],
                                   ps[0:64, :qn])
                    nc.scalar.copy(kz_sb[2 * pair + 1][64:128, q0:q0 + qn],
                                   ps[64:128, :qn])

    wqkpool.release()
    wvpool.release()
    xpool.release()

    if dbg is not None:
        for pair in range(6):
            nc.sync.dma_start(dbg["qk"][pair], q_sb[pair][:, :])
        for h in range(6):
            nc.sync.dma_start(dbg["qk"][6 + h], kz_sb[h][:, :])
        for kt in range(NKT):
            nc.sync.dma_start(
                dbg["v"][kt], v_sb[kt][:, :, :].rearrange("p h d -> p (h d)"))

    # ---------------- phase 2: attention, one head at a time -----------------
    ppool = ctx.enter_context(tc.tile_pool(name="ppool", bufs=LAG0 + 2))
    p0pool = ctx.enter_context(tc.tile_pool(name="p0pool", bufs=2))
    ptpool = ctx.enter_context(tc.tile_pool(name="ptpool", bufs=2))
    npool = ctx.enter_context(tc.tile_pool(name="npool", bufs=2))
    ocpool = ctx.enter_context(tc.tile_pool(name="ocpool", bufs=2))

    bts = {}

    def issue_bdma(u):
        if u >= 12 * NKT:
            return
        h, kt = divmod(u, NKT)
        p = ktsize(kt)
        bt = bpool.tile([128, 1024], bf16, tag="bt", name=f"bt_{h}_{kt}")
        nc.sync.dma_start(bt[:p, :], expb[h, kt, 0:p, :])
        bts[u] = bt

    for u in range(LAGB):
        issue_bdma(u)

    # PSUM budget (8 banks): S tiles 2x[128,1024] = 4, o_ps 2x[65,512] = 2,
    # tail tiles 2x[128,48] = 2.
    def make_norm_stage(pair, pr0, ocp, rsm, q0, qn, h, ci):
        # reciprocal + broadcast + scale, reading the SBUF evacuation of
        # o_ps; deferred into the NEXT head's units so the DVE queue always
        # serves the next head's P multiplies first. rsm must be a
        # partition-0 SBUF tile (custom-DVE reciprocal and the gpsimd
        # partition broadcast both only read physical partition 0).
        def stage():
            rs = npool.tile([1, 512], f32, tag="rs", name=f"rs_{h}_{ci}")
            nc.vector.reciprocal_approx_fast(rs[:, :qn], rsm[:, :qn])
            bc = npool.tile([64, 512], f32, tag="bc", name=f"bc_{h}_{ci}")
            nc.gpsimd.partition_broadcast(bc[:, :qn], rs[:, :qn])
            nc.vector.tensor_mul(
                o_all[pair][pr0:pr0 + 64, q0:q0 + qn],
                ocp[0:64, :qn], bc[:, :qn])
        return stage

    with tc.tile_pool(name="ps_s", bufs=2, space="PSUM") as ps_s, \
         tc.tile_pool(name="ps_o", bufs=1, space="PSUM") as ps_o, \
         tc.tile_pool(name="ps_t", bufs=2, space="PSUM") as ps_t:

        pending = []
        for h in range(12):
            pair, hh = divmod(h, 2)
            pr0 = hh * 64
            q_t = q_sb[pair]
            k_t = kz_sb[h]
            lag = LAG0 if h == 0 else LAG

            o_ps = [
                ps_o.tile([65, 512], f32, tag="o0", name=f"o0_{h}"),
                ps_o.tile([65, 512], f32, tag="o1", name=f"o1_{h}"),
            ]
            # shared tail bank: S tail cols [0:36], O tail accum [0:65, 40:44]
            st = ps_t.tile([128, 48], f32, tag="st", name=f"st_{h}")

            btl = btail_pool.tile([128, 36], bf16, tag="btl", name=f"btl_{h}")
            nc.sync.dma_start(btl[:, :], expb_tail[h, :, :])

            pes = {}

            def pv_unit(kt):
                # K=128 always: v_sb[8] rows past the real token are zeroed,
                # so stale (finite) pe rows contribute nothing.
                pe = pes.pop(kt)
                for ci, (q0, qn) in enumerate(QCHUNKS):
                    nc.tensor.matmul(
                        o_ps[ci][:, :],
                        v_sb[kt][:, h, :],
                        pe[:, q0:q0 + qn],
                        start=(kt == 0), stop=(kt == 8),
                    )

            for kt in range(NKT):
                issue_bdma(h * NKT + kt + LAGB)
                p = ktsize(kt)
                s2 = ps_s.tile([128, 1024], f32, tag="s2", name=f"s2_{h}_{kt}")
                for (q0, qn) in QCHUNKS:
                    nc.tensor.matmul(
                        s2[:p, q0:q0 + qn],
                        k_t[:, kt * 128: kt * 128 + p],
                        q_t[:, q0:q0 + qn],
                        start=True, stop=True,
                    )
                nc.tensor.matmul(
                    st[:p, 4 * kt:4 * kt + 4],
                    k_t[:, kt * 128: kt * 128 + p],
                    q_t[:, 1024:1028],
                    start=True, stop=True,
                )
                if kt == 0:
                    nc.vector.memset(st[0:65, 40:44], 0.0)
                pe0 = p0pool.tile([128, 1024], bf16, tag="pe0",
                                  name=f"pe0_{h}_{kt}")
                nc.scalar.activation(pe0[:p, :], s2[:p, :], AF.Exp)
                pe = ppool.tile([128, 1024], bf16, tag="pe",
                                name=f"pe_{h}_{kt}")
                nc.vector.tensor_mul(pe[:p, :], pe0[:p, :],
                                     bts[h * NKT + kt][:p, :])
                pes[kt] = pe
                if dbg is not None and h == 0:
                    nc.sync.dma_start(dbg["pe"][kt, 0:p, :], pe[:p, :])
                if pending:
                    pending.pop(0)()
                if kt >= lag:
                    pv_unit(kt - lag)
            for kt in range(NKT - lag, NKT):
                pv_unit(kt)

            # evacuate O + rowsum to SBUF immediately - this frees the o_ps
            # PSUM banks so the next head's PV can start; normalize math is
            # deferred into the next head's units. rowsum rows go to
            # partition-0 tiles (custom-DVE reciprocal requirement), with
            # the copies split across DVE and ACT for balance.
            ocp0 = ocpool.tile([65, 512], f32, tag="ocp0", name=f"ocp0_{h}")
            nc.vector.tensor_copy(ocp0[:, :], o_ps[0][:, :])
            rsm0 = ocpool.tile([1, 512], f32, tag="rsm0", name=f"rsm0_{h}")
            nc.scalar.copy(rsm0[:, :], o_ps[0][64:65, :])
            ocp1 = ocpool.tile([65, 512], f32, tag="ocp1", name=f"ocp1_{h}")
            nc.scalar.copy(ocp1[:, :], o_ps[1][:, :])
            rsm1 = ocpool.tile([1, 512], f32, tag="rsm1", name=f"rsm1_{h}")
            nc.vector.tensor_copy(rsm1[:, :], o_ps[1][64:65, :])

            # q-tail (token 1024): one exp+mult for all 9 k tiles, then the
            # 9 tiny PV matmuls accumulate into the shared tail bank region.
            pt0 = ptpool.tile([128, 36], bf16, tag="pt0", name=f"pt0_{h}")
            nc.scalar.activation(pt0[:, :], st[:, 0:36], AF.Exp)
            ptl = ptpool.tile([128, 36], bf16, tag="ptl", name=f"ptl_{h}")
            nc.vector.tensor_mul(ptl[:, :], pt0[:, :], btl[:, :])
            for kt in range(NKT):
                nc.tensor.matmul(
                    st[0:65, 40:44],
                    v_sb[kt][:, h, :],
                    ptl[:, 4 * kt:4 * kt + 4],
                    start=False, stop=(kt == 8),
                    skip_group_check=True,
                )
            ocpt = ocpool.tile([65, 4], f32, tag="ocpt", name=f"ocpt_{h}")
            nc.vector.tensor_copy(ocpt[:, :], st[0:65, 40:44])
            rsmt = ocpool.tile([1, 4], f32, tag="rsmt", name=f"rsmt_{h}")
            nc.vector.tensor_copy(rsmt[:, :], st[64:65, 40:44])
            if dbg is not None and h == 0:
                nc.sync.dma_start(dbg["ops"][0], ocp0[:, :])
                nc.sync.dma_start(dbg["ops"][1], ocp1[:, :])

            pending.append(make_norm_stage(pair, pr0, ocp0, rsm0, 0, 512, h, 0))
            pending.append(make_norm_stage(pair, pr0, ocp1, rsm1, 512, 512, h, 1))
            pending.append(make_norm_stage(pair, pr0, ocpt, rsmt, 1024, 4, h, 2))
        for s in pending:
            s()

    if dbg is not None:
        for m in range(6):
            nc.sync.dma_start(dbg["oall"][m], o_all[m][:, :])

    # ---------------- phase 3: output projection ----------------
    # ct-outer/chunk-inner so consecutive matmuls share a wproj stationary.
    with tc.tile_pool(name="ops3", bufs=2, space="PSUM") as ps3, \
         tc.tile_pool(name="out_pool", bufs=3) as out_pool:
        for m in range(6):
            pss = [ps3.tile([128, 512], f32, tag=f"ps3_{ci}",
                            name=f"ps3_{m}_{ci}")
                   for ci in range(3)]
            for ct in range(6):
                for ci, (q0, qn) in enumerate(QCHUNKS3):
                    nc.tensor.matmul(
                        pss[ci][:, :qn],
                        wproj_t[ct][:, m * 128:(m + 1) * 128],
                        o_all[ct][:, q0:q0 + qn],
                        start=(ct == 0), stop=(ct == 5),
                    )
            for ci, (q0, qn) in enumerate(QCHUNKS3):
                wn = min(qn, N - q0)
                ot = out_pool.tile([128, 512], f32, tag="ot",
                                   name=f"ot_{m}_{q0}")
                nc.scalar.activation(ot[:, :wn], pss[ci][:, :wn], AF.Identity,
                                     bias=projb_t[m])
                nc.sync.dma_start(outT[m * 128:(m + 1) * 128, q0:q0 + wn],
                                  ot[:, :wn])


def _host_prep(x, qkv_w, q_bias, v_bias, rpb_table, proj_w, proj_b,
               rel_pos_index):
    """Layout transforms + exact bias folds; bf16 casts for matmul inputs."""
    import ml_dtypes
    f = np.float32
    bf = ml_dtypes.bfloat16
    x = np.asarray(x, f)
    qkv_w = np.asarray(qkv_w, f)
    q_bias = np.asarray(q_bias, f)
    v_bias = np.asarray(v_bias, f)
    rpb_table = np.asarray(rpb_table, f)
    proj_w = np.asarray(proj_w, f)
    proj_b = np.asarray(proj_b, f)
    idx = np.asarray(rel_pos_index)

    # q/k weights: column blocks [q0 q1 | k0 k1 | q2 q3 | k2 k3 | ...],
    # q pre-scaled by 1/sqrt(D); q/k biases added at PSUM evacuation.
    rows = []
    brows = []
    for p in range(6):
        rows.append(qkv_w[p * 128:(p + 1) * 128] * SCALE)
        brows.append(q_bias[p * 128:(p + 1) * 128] * SCALE)
        rows.append(qkv_w[C + p * 128: C + (p + 1) * 128])
        brows.append(np.zeros(128, f))
    wqk = np.concatenate(rows, axis=0)               # [1536, 768]
    wqk_bias = np.concatenate(brows, axis=0)         # [1536]
    wqk_aug = np.ascontiguousarray(wqk.T).astype(bf)  # [768, 1536]
    wqkb = np.ascontiguousarray(wqk_bias.reshape(12, 128).T)  # [128, 12] f32

    wv = np.ascontiguousarray(qkv_w[2 * C:3 * C].T).astype(bf)  # [768, 768]

    rpb = rpb_table[idx]                              # [N, N, H] (q, k, h)
    biasT = np.ascontiguousarray(rpb.transpose(2, 1, 0))  # [H, k, q]
    expb_pad = np.zeros((H, NKT * 128, NP), f)
    expb_pad[:, :N, :N] = biasT
    expb_pad = np.exp(expb_pad)                       # pads exp(0) = 1
    expb = np.ascontiguousarray(
        expb_pad[:, :, :1024].reshape(H, NKT, 128, 1024)).astype(bf)
    # tail: q cols 1024:1028 for each k tile, packed [H, 128, 4*NKT]
    tail = expb_pad[:, :, 1024:1028].reshape(H, NKT, 128, 4)
    expb_tail = np.ascontiguousarray(
        tail.transpose(0, 2, 1, 3).reshape(H, 128, 4 * NKT)).astype(bf)

    wproj = np.ascontiguousarray(proj_w.T).astype(bf)  # [768, 768]
    # v_bias folded exactly into the projection bias:
    # out += v_bias @ proj_w.T == proj_w @ v_bias (per output channel)
    projb_eff = proj_b + proj_w @ v_bias
    projb = np.ascontiguousarray(projb_eff.reshape(C, 1)).astype(f)

    xT = np.zeros((B, C, NP), bf)
    for b in range(B):
        xT[b, :, :N] = x[b].T.astype(bf)
    return xT, wqk_aug, wqkb, wv, expb, expb_tail, wproj, projb


def _dedupe_ldweights(nc):
    """Remove InstLdweights that reload the identical stationary AP as the
    immediately preceding PE ldweights (only matmuls in between, which leave
    the loaded weights intact). The duplicate's dependants are remapped to
    the surviving ldweights; incoming deps are merged (identical anyway
    since the stationary AP is identical)."""
    import concourse.mybir as mybir
    PE = mybir.EngineType.PE
    n_del = 0
    for blk in nc.main_func.blocks:
        keep = []
        last_key = None
        last_ldw = None
        for inst in blk.instructions:
            tn = type(inst).__name__
            if getattr(inst, "engine", None) == PE:
                if tn == "InstLdweights":
                    key = str(inst.ins[0])
                    si = inst.sync_info
                    clean = si is None or (
                        len(si.on_wait) == 0 and len(si.on_update) == 0)
                    if key == last_key and clean and last_ldw is not None:
                        for dname in list(inst.descendants or []):
                            dep = nc.inst_map.get(dname)
                            if dep is not None:
                                try:
                                    dep.remap_dependency_names(
                                        {inst.name: last_ldw.name})
                                except Exception:
                                    pass
                        try:
                            last_ldw.merge_dependencies_from(inst)
                        except Exception:
                            pass
                        n_del += 1
                        continue
                    last_key = key
                    last_ldw = inst
                elif tn != "InstMatmult":
                    last_key = None
                    last_ldw = None
            keep.append(inst)
        blk.instructions[:] = keep
    return n_del


_BUILT = {}


def _build():
    if "nc" in _BUILT:
        return _BUILT["nc"]
    from contextlib import ExitStack

    import concourse.mybir as mybir
    import concourse.tile as tile
    from concourse import bacc

    nc = bacc.Bacc("TRN2", target_bir_lowering=False, debug=False,
                   num_devices=B)
    f32 = mybir.dt.float32
    bf16 = mybir.dt.bfloat16
    xT = nc.dram_tensor("xT", (C, NP), bf16, kind="ExternalInput").ap()
    wqk = nc.dram_tensor("wqk", (768, 1536), bf16, kind="ExternalInput").ap()
    wqkb = nc.dram_tensor("wqkb", (128, 12), f32, kind="ExternalInput").ap()
    wv = nc.dram_tensor("wv", (768, 768), bf16, kind="ExternalInput").ap()
    expb = nc.dram_tensor("expb", (H, NKT, 128, 1024), bf16,
                          kind="ExternalInput").ap()
    expb_tail = nc.dram_tensor("expb_tail", (H, 128, 4 * NKT), bf16,
                               kind="ExternalInput").ap()
    wproj = nc.dram_tensor("wproj", (768, 768), bf16,
                           kind="ExternalInput").ap()
    projb = nc.dram_tensor("projb", (768, 1), f32, kind="ExternalInput").ap()
    outT = nc.dram_tensor("outT", (768, N), f32, kind="ExternalOutput").ap()

    dbg = None
    if DEBUG:
        dbg = {
            "qk": nc.dram_tensor("dbg_qk", (12, 128, NP), bf16,
                                 kind="ExternalOutput").ap(),
            "v": nc.dram_tensor("dbg_v", (NKT, 128, H * (D + 1)), bf16,
                                kind="ExternalOutput").ap(),
            "pe": nc.dram_tensor("dbg_pe", (NKT, 128, 1024), bf16,
                                 kind="ExternalOutput").ap(),
            "ops": nc.dram_tensor("dbg_ops", (2, 65, 512), f32,
                                  kind="ExternalOutput").ap(),
            "rs": nc.dram_tensor("dbg_rs", (2, 1, 512), f32,
                                 kind="ExternalOutput").ap(),
            "bc": nc.dram_tensor("dbg_bc", (2, 64, 512), f32,
                                 kind="ExternalOutput").ap(),
            "oall": nc.dram_tensor("dbg_oall", (6, 128, NP), bf16,
                                   kind="ExternalOutput").ap(),
        }

    with tile.TileContext(nc) as tc:
        with ExitStack() as ctx:
            _emit(ctx, tc, xT, wqk, wqkb, wv, expb, expb_tail, wproj,
                  projb, outT, dbg=dbg)
    _dedupe_ldweights(nc)
    nc.compile()
    _BUILT["nc"] = nc
    return nc


def kernel(x, qkv_w, q_bias, v_bias, rpb_table, proj_w, proj_b,
           rel_pos_index):
    from concourse.bass_utils import run_bass_kernel_spmd

    xT, wqk, wqkb, wv, expb, expb_tail, wproj, projb = _host_prep(
        x, qkv_w, q_bias, v_bias, rpb_table, proj_w, proj_b, rel_pos_index)

    nc = _build()
    shared = {
        "wqk": wqk, "wqkb": wqkb, "wv": wv, "expb": expb,
        "expb_tail": expb_tail, "wproj": wproj, "projb": projb,
    }
    in_maps = [dict(shared, xT=np.ascontiguousarray(xT[b]))
               for b in range(B)]
    res = run_bass_kernel_spmd(nc, in_maps, core_ids=list(range(B)))
    out = np.stack([res.results[b]["outT"].T for b in range(B)], axis=0)
    return out.astype(np.float32)


# revision 47
# speedup vs baseline: 1.0678x; 1.0175x over previous
"""Trainium2 Bass kernel for Swin-style windowed attention w/ relative position bias.

Problem: x[8, 1025, 768], 12 heads, head_dim 64, rel-pos bias table gathered
by a constant index matrix. Sharding: pure data-parallel - one batch element
per NeuronCore (8 cores).

v2 dataflow (all matmuls bf16; fp32 PSUM accumulation):
  xT [768, 1028]          x[b].T, bf16
  qkT = wqk.T @ xT        -> [1536, 1028] (q/k per head pair, q pre-scaled)
  v   = xT.T @ wv         -> [1025, 768]  (tokens-major, ones col for rowsum)
  S.T[k,q] = k_h @ q_h.T  (K=64, PSUM tile [128,1024] spanning 2 banks,
                           q tail col 1024 in a shared tail bank)
  P.T = exp(S.T) * expB   (one wide ACT exp evacuates PSUM -> bf16; bias is
                           applied multiplicatively: expB = exp(bias) is
                           precomputed on host and streamed bf16, so the DVE
                           multiply runs on all-bf16 SBUF operands)
  [O.T; rowsum] = [v_h|1].T @ P.T
  O.T *= rcp(rowsum)      (DVE fast reciprocal + gpsimd partition broadcast;
                           no Ln/table switches on ACT - one act table total)
  outT = wproj.T @ O_all + projb_eff,  projb_eff = proj_b + proj_w @ v_bias
                           (v_bias folded exactly into the proj bias on host)
"""

import sys

import numpy as np

for _p in ("/opt/trn_rl_repo",):
    if _p not in sys.path:
        sys.path.insert(0, _p)

B = 8
N = 1025
NP = 1028        # q padded: 2 full PSUM banks + 4-wide tail
C = 768
H = 12
D = 64
SCALE = D ** -0.5
NKT = 9          # k tiles of 128 (8 full + 1)
QCHUNKS = [(0, 512), (512, 512)]          # main q chunks (tail separate)
QCHUNKS3 = [(0, 512), (512, 512), (1024, 4)]
NCHUNKS_V = [(0, 512), (512, 256)]
LAG = 4          # S->PV software pipeline depth (units); deeper for head 0
LAG0 = 6         # (absorbs the phase1->phase2 engine-queue debt)
LAGB = 3         # expB DMA prefetch depth (units)
DEBUG = False    # set by debug harness; adds intermediate dump outputs


def _emit(ctx, tc, xT, wqk, wqkb, wv, expb, expb_tail, wproj, projb, outT,
          dbg=None):
    import concourse.mybir as mybir

    nc = tc.nc
    f32 = mybir.dt.float32
    bf16 = mybir.dt.bfloat16
    AF = mybir.ActivationFunctionType

    def ktsize(kt):
        return 128 if kt < 8 else 1

    lp = nc.allow_low_precision(
        reason="bf16 matmul inputs / elementwise; f32 PSUM accumulation")
    lp.__enter__()
    ctx.callback(lambda: lp.__exit__(None, None, None))

    # Long-lived pools first (pool release must be LIFO / stack ordered).
    # The PE HAM clock gate only un-throttles (1.2 -> 2.4 GHz) for matmuls
    # whose stationary spans all 128 K rows; K=64 matmuls anywhere in the
    # stream keep the core cold. So every matmul here is K=128: the S
    # stationaries are per-head zero-padded k tiles (kz), and v_sb[8]'s
    # invalid rows are zeroed so the kt=8 PV can contract over 128 rows.
    qk_pool = ctx.enter_context(tc.tile_pool(name="qk_pool", bufs=1))
    v_pool = ctx.enter_context(tc.tile_pool(name="v_pool", bufs=1))
    q_sb = []
    for pair in range(6):
        t = qk_pool.tile([128, NP], bf16, tag=f"q{pair}", name=f"q{pair}")
        q_sb.append(t)
    kz_sb = []
    for h in range(12):
        t = qk_pool.tile([128, NP], bf16, tag=f"kz{h}", name=f"kz{h}")
        kz_sb.append(t)
        # zero the opposite head's 64 rows (once; never overwritten)
        nc.gpsimd.memset(t[(1 - h % 2) * 64:(2 - h % 2) * 64, :], 0.0)
    v_sb = []
    for kt in range(NKT):
        t = v_pool.tile([128, H, D + 1], bf16, tag=f"v{kt}", name=f"v{kt}")
        v_sb.append(t)
        if kt < 8:
            nc.vector.memset(t[:, :, D:D + 1], 1.0)
        else:
            # only row 0 is a real token; zero the rest so K=128 PV is exact
            nc.vector.memset(t[:, :, :], 0.0)
            nc.vector.memset(t[0:1, :, D:D + 1], 1.0)

    opool = ctx.enter_context(tc.tile_pool(name="opool", bufs=1))
    o_all = []
    for m in range(6):
        t = opool.tile([128, NP], bf16, tag=f"oall{m}", name=f"oall{m}")
        o_all.append(t)

    wp_pool = ctx.enter_context(tc.tile_pool(name="wp_pool", bufs=1))
    wproj_t = []
    projb_t = []
    for ct in range(6):
        t = wp_pool.tile([128, 768], bf16, tag=f"wproj{ct}", name=f"wproj{ct}")
        wproj_t.append(t)
        tb = wp_pool.tile([128, 1], f32, tag=f"projb{ct}", name=f"projb{ct}")
        projb_t.append(tb)

    # expB stream pools (addresses must not overlap phase-1 tiles so the
    # DMAs can prefetch during the QKV projections).
    bpool = ctx.enter_context(tc.tile_pool(name="bpool", bufs=LAGB + 2))
    btail_pool = ctx.enter_context(tc.tile_pool(name="btail", bufs=2))

    # ---------------- phase 1: load weights + x, QKV projections -------------
    xpool = tc.alloc_tile_pool(name="xpool", bufs=1)
    wvpool = tc.alloc_tile_pool(name="wvpool", bufs=1)
    wqkpool = tc.alloc_tile_pool(name="wqkpool", bufs=1)

    # DMA issue order matters: x + wv first (v matmuls unblock soonest),
    # wqk after (only needed ~20us in), wproj/projb last (phase 3).
    wqk_t, wv_t, xT_t = [], [], []
    for ct in range(6):
        xt = xpool.tile([128, NP], bf16, tag=f"xT{ct}", name=f"xT{ct}")
        nc.sync.dma_start(xt[:, :], xT[ct * 128:(ct + 1) * 128, :])
        xT_t.append(xt)
        w2 = wvpool.tile([128, 768], bf16, tag=f"wv{ct}", name=f"wv{ct}")
        nc.sync.dma_start(w2[:, :], wv[ct * 128:(ct + 1) * 128, :])
        wv_t.append(w2)
        w1 = wqkpool.tile([128, 1536], bf16, tag=f"wqk{ct}", name=f"wqk{ct}")
        wqk_t.append(w1)
    wqkb_t = wqkpool.tile([128, 12], f32, tag="wqkb", name="wqkb")
    for ct in range(6):
        nc.sync.dma_start(wqk_t[ct][:, :], wqk[ct * 128:(ct + 1) * 128, :])
    nc.sync.dma_start(wqkb_t[:, :], wqkb[:, :])
    for ct in range(6):
        nc.sync.dma_start(wproj_t[ct][:, :], wproj[ct * 128:(ct + 1) * 128, :])
        nc.sync.dma_start(projb_t[ct][:, :], projb[ct * 128:(ct + 1) * 128, :])

    with tc.tile_pool(name="ps1", bufs=4, space="PSUM") as ps1, \
         tc.tile_pool(name="ps1v", bufs=2, space="PSUM") as ps1v:
        # v first (needs only xT+wv): PE starts sooner. ct-outer/chunk-inner
        # so consecutive matmuls share a stationary (ldweights dedupe).
        for kt in range(NKT):
            p = ktsize(kt)
            ps = ps1v.tile([128, 768], f32, tag="ps1v", name=f"ps1v_{kt}")
            for ct in range(6):
                for (c0, cn) in NCHUNKS_V:
                    nc.tensor.matmul(
                        ps[:p, c0:c0 + cn],
                        xT_t[ct][:, kt * 128: kt * 128 + p],
                        wv_t[ct][:, c0:c0 + cn],
                        start=(ct == 0), stop=(ct == 5),
                    )
            nc.scalar.copy(
                v_sb[kt][:p, :, 0:D],
                ps[:p, :].rearrange("p (h d) -> p h d", h=H),
            )
        # q/k: qkT[m-tile, n] = sum_ct wqk[ct, m-tile].T @ xT[ct, n]
        # q m-tiles (even) evacuate whole; k m-tiles (odd) split per head
        # into the zero-padded kz stationaries.
        for m in range(12):
            pair = m // 2
            pss = [ps1.tile([128, 512], f32, tag="ps1t", name=f"ps1_{m}_{q0}")
                   for (q0, qn) in QCHUNKS3]
            for ct in range(6):
                for ci, (q0, qn) in enumerate(QCHUNKS3):
                    nc.tensor.matmul(
                        pss[ci][:, :qn],
                        wqk_t[ct][:, m * 128:(m + 1) * 128],
                        xT_t[ct][:, q0:q0 + qn],
                        start=(ct == 0), stop=(ct == 5),
                    )
            for ci, (q0, qn) in enumerate(QCHUNKS3):
                ps = pss[ci]
                if m % 2 == 0:
                    nc.scalar.activation(q_sb[pair][:, q0:q0 + qn], ps[:, :qn],
                                         AF.Identity, bias=wqkb_t[:, m:m + 1])
                else:
                    nc.scalar.copy(kz_sb[2 * pair][0:64, q0:q0 + qn],
                                   ps[0:64, :qn])
                    nc.scalar.copy(kz_sb[2 * pair + 1][64:128, q0:q0 + qn],
                                   ps[64:128, :qn])

    wqkpool.release()
    wvpool.release()
    xpool.release()

    if dbg is not None:
        for pair in range(6):
            nc.sync.dma_start(dbg["qk"][pair], q_sb[pair][:, :])
        for h in range(6):
            nc.sync.dma_start(dbg["qk"][6 + h], kz_sb[h][:, :])
        for kt in range(NKT):
            nc.sync.dma_start(
                dbg["v"][kt], v_sb[kt][:, :, :].rearrange("p h d -> p (h d)"))

    # ---------------- phase 2: attention, one head at a time -----------------
    ppool = ctx.enter_context(tc.tile_pool(name="ppool", bufs=LAG0 + 2))
    p0pool = ctx.enter_context(tc.tile_pool(name="p0pool", bufs=2))
    ptpool = ctx.enter_context(tc.tile_pool(name="ptpool", bufs=2))
    npool = ctx.enter_context(tc.tile_pool(name="npool", bufs=2))
    ocpool = ctx.enter_context(tc.tile_pool(name="ocpool", bufs=2))

    bts = {}

    def issue_bdma(u):
        if u >= 12 * NKT:
            return
        h, kt = divmod(u, NKT)
        p = ktsize(kt)
        bt = bpool.tile([128, 1024], bf16, tag="bt", name=f"bt_{h}_{kt}")
        nc.sync.dma_start(bt[:p, :], expb[h, kt, 0:p, :])
        bts[u] = bt

    for u in range(LAGB):
        issue_bdma(u)

    # PSUM budget (8 banks): S tiles 2x[128,1024] = 4, o_ps 2x[65,512] = 2,
    # tail tiles 2x[128,48] = 2.
    def make_norm_stage(pair, pr0, ocp, rsm, q0, qn, h, ci):
        # reciprocal + broadcast + scale, reading the SBUF evacuation of
        # o_ps; deferred into the NEXT head's units so the DVE queue always
        # serves the next head's P multiplies first. rsm must be a
        # partition-0 SBUF tile (custom-DVE reciprocal and the gpsimd
        # partition broadcast both only read physical partition 0).
        def stage():
            rs = npool.tile([1, 512], f32, tag="rs", name=f"rs_{h}_{ci}")
            nc.vector.reciprocal_approx_fast(rs[:, :qn], rsm[:, :qn])
            bc = npool.tile([64, 512], f32, tag="bc", name=f"bc_{h}_{ci}")
            nc.gpsimd.partition_broadcast(bc[:, :qn], rs[:, :qn])
            nc.vector.tensor_mul(
                o_all[pair][pr0:pr0 + 64, q0:q0 + qn],
                ocp[0:64, :qn], bc[:, :qn])
        return stage

    with tc.tile_pool(name="ps_s", bufs=2, space="PSUM") as ps_s, \
         tc.tile_pool(name="ps_o", bufs=1, space="PSUM") as ps_o, \
         tc.tile_pool(name="ps_t", bufs=2, space="PSUM") as ps_t:

        pending = []
        for h in range(12):
            pair, hh = divmod(h, 2)
            pr0 = hh * 64
            q_t = q_sb[pair]
            k_t = kz_sb[h]
            lag = LAG0 if h == 0 else LAG

            o_ps = [
                ps_o.tile([65, 512], f32, tag="o0", name=f"o0_{h}"),
                ps_o.tile([65, 512], f32, tag="o1", name=f"o1_{h}"),
            ]
            # shared tail bank: S tail cols [0:36], O tail accum [0:65, 40:44]
            st = ps_t.tile([128, 48], f32, tag="st", name=f"st_{h}")

            btl = btail_pool.tile([128, 36], bf16, tag="btl", name=f"btl_{h}")
            nc.sync.dma_start(btl[:, :], expb_tail[h, :, :])

            pes = {}

            def pv_unit(kt):
                # K=128 always: v_sb[8] rows past the real token are zeroed,
                # so stale (finite) pe rows contribute nothing.
                pe = pes.pop(kt)
                for ci, (q0, qn) in enumerate(QCHUNKS):
                    nc.tensor.matmul(
                        o_ps[ci][:, :],
                        v_sb[kt][:, h, :],
                        pe[:, q0:q0 + qn],
                        start=(kt == 0), stop=(kt == 8),
                    )

            for kt in range(NKT):
                issue_bdma(h * NKT + kt + LAGB)
                p = ktsize(kt)
                # PV first: its inputs are lag units old and always ready,
                # so PE has work even when exp/mult trail the S pipeline.
                if kt >= lag:
                    pv_unit(kt - lag)
                s2 = ps_s.tile([128, 1024], f32, tag="s2", name=f"s2_{h}_{kt}")
                for (q0, qn) in QCHUNKS:
                    nc.tensor.matmul(
                        s2[:p, q0:q0 + qn],
                        k_t[:, kt * 128: kt * 128 + p],
                        q_t[:, q0:q0 + qn],
                        start=True, stop=True,
                    )
                nc.tensor.matmul(
                    st[:p, 4 * kt:4 * kt + 4],
                    k_t[:, kt * 128: kt * 128 + p],
                    q_t[:, 1024:1028],
                    start=True, stop=True,
                )
                if kt == 0:
                    nc.vector.memset(st[0:65, 40:44], 0.0)
                pe0 = p0pool.tile([128, 1024], bf16, tag="pe0",
                                  name=f"pe0_{h}_{kt}")
                nc.scalar.activation(pe0[:p, :], s2[:p, :], AF.Exp)
                pe = ppool.tile([128, 1024], bf16, tag="pe",
                                name=f"pe_{h}_{kt}")
                nc.vector.tensor_mul(pe[:p, :], pe0[:p, :],
                                     bts[h * NKT + kt][:p, :])
                pes[kt] = pe
                if dbg is not None and h == 0:
                    nc.sync.dma_start(dbg["pe"][kt, 0:p, :], pe[:p, :])
                if pending:
                    pending.pop(0)()
            for kt in range(NKT - lag, NKT):
                pv_unit(kt)

            # evacuate O + rowsum to SBUF immediately - this frees the o_ps
            # PSUM banks so the next head's PV can start; normalize math is
            # deferred into the next head's units. rowsum rows go to
            # partition-0 tiles (custom-DVE reciprocal requirement), with
            # the copies split across DVE and ACT for balance.
            ocp0 = ocpool.tile([65, 512], f32, tag="ocp0", name=f"ocp0_{h}")
            nc.vector.tensor_copy(ocp0[:, :], o_ps[0][:, :])
            rsm0 = ocpool.tile([1, 512], f32, tag="rsm0", name=f"rsm0_{h}")
            nc.scalar.copy(rsm0[:, :], o_ps[0][64:65, :])
            ocp1 = ocpool.tile([65, 512], f32, tag="ocp1", name=f"ocp1_{h}")
            nc.scalar.copy(ocp1[:, :], o_ps[1][:, :])
            rsm1 = ocpool.tile([1, 512], f32, tag="rsm1", name=f"rsm1_{h}")
            nc.vector.tensor_copy(rsm1[:, :], o_ps[1][64:65, :])
            if dbg is not None and h == 0:
                nc.sync.dma_start(dbg["ops"][0], ocp0[:, :])
                nc.sync.dma_start(dbg["ops"][1], ocp1[:, :])

            def make_tail_stage(pair, pr0, st, btl, h):
                # q-tail (token 1024): one exp+mult for all 9 k tiles, the 9
                # tiny PV matmuls, evacuation, and its normalize. Deferred
                # into the next head's first unit so the small matmuls fill
                # the PE while the next head's first exp is still cooking.
                def stage():
                    pt0 = ptpool.tile([128, 36], bf16, tag="pt0",
                                      name=f"pt0_{h}")
                    nc.scalar.activation(pt0[:, :], st[:, 0:36], AF.Exp)
                    ptl = ptpool.tile([128, 36], bf16, tag="ptl",
                                      name=f"ptl_{h}")
                    nc.vector.tensor_mul(ptl[:, :], pt0[:, :], btl[:, :])
                    for kt in range(NKT):
                        nc.tensor.matmul(
                            st[0:65, 40:44],
                            v_sb[kt][:, h, :],
                            ptl[:, 4 * kt:4 * kt + 4],
                            start=False, stop=(kt == 8),
                            skip_group_check=True,
                        )
                    ocpt = ocpool.tile([65, 4], f32, tag="ocpt",
                                       name=f"ocpt_{h}")
                    nc.vector.tensor_copy(ocpt[:, :], st[0:65, 40:44])
                    rsmt = ocpool.tile([1, 4], f32, tag="rsmt",
                                       name=f"rsmt_{h}")
                    nc.vector.tensor_copy(rsmt[:, :], st[64:65, 40:44])
                    make_norm_stage(pair, pr0, ocpt, rsmt, 1024, 4, h, 2)()
                return stage

            pending.append(make_tail_stage(pair, pr0, st, btl, h))
            pending.append(make_norm_stage(pair, pr0, ocp0, rsm0, 0, 512, h, 0))
            pending.append(make_norm_stage(pair, pr0, ocp1, rsm1, 512, 512, h, 1))
        for s in pending:
            s()

    if dbg is not None:
        for m in range(6):
            nc.sync.dma_start(dbg["oall"][m], o_all[m][:, :])

    # ---------------- phase 3: output projection ----------------
    # ct-outer/chunk-inner so consecutive matmuls share a wproj stationary.
    with tc.tile_pool(name="ops3", bufs=2, space="PSUM") as ps3, \
         tc.tile_pool(name="out_pool", bufs=3) as out_pool:
        for m in range(6):
            pss = [ps3.tile([128, 512], f32, tag=f"ps3_{ci}",
                            name=f"ps3_{m}_{ci}")
                   for ci in range(3)]
            for ct in range(6):
                for ci, (q0, qn) in enumerate(QCHUNKS3):
                    nc.tensor.matmul(
                        pss[ci][:, :qn],
                        wproj_t[ct][:, m * 128:(m + 1) * 128],
                        o_all[ct][:, q0:q0 + qn],
                        start=(ct == 0), stop=(ct == 5),
                    )
            for ci, (q0, qn) in enumerate(QCHUNKS3):
                wn = min(qn, N - q0)
                ot = out_pool.tile([128, 512], f32, tag="ot",
                                   name=f"ot_{m}_{q0}")
                nc.scalar.activation(ot[:, :wn], pss[ci][:, :wn], AF.Identity,
                                     bias=projb_t[m])
                nc.sync.dma_start(outT[m * 128:(m + 1) * 128, q0:q0 + wn],
                                  ot[:, :wn])


def _host_prep(x, qkv_w, q_bias, v_bias, rpb_table, proj_w, proj_b,
               rel_pos_index):
    """Layout transforms + exact bias folds; bf16 casts for matmul inputs."""
    import ml_dtypes
    f = np.float32
    bf = ml_dtypes.bfloat16
    x = np.asarray(x, f)
    qkv_w = np.asarray(qkv_w, f)
    q_bias = np.asarray(q_bias, f)
    v_bias = np.asarray(v_bias, f)
    rpb_table = np.asarray(rpb_table, f)
    proj_w = np.asarray(proj_w, f)
    proj_b = np.asarray(proj_b, f)
    idx = np.asarray(rel_pos_index)

    # q/k weights: column blocks [q0 q1 | k0 k1 | q2 q3 | k2 k3 | ...],
    # q pre-scaled by 1/sqrt(D); q/k biases added at PSUM evacuation.
    rows = []
    brows = []
    for p in range(6):
        rows.append(qkv_w[p * 128:(p + 1) * 128] * SCALE)
        brows.append(q_bias[p * 128:(p + 1) * 128] * SCALE)
        rows.append(qkv_w[C + p * 128: C + (p + 1) * 128])
        brows.append(np.zeros(128, f))
    wqk = np.concatenate(rows, axis=0)               # [1536, 768]
    wqk_bias = np.concatenate(brows, axis=0)         # [1536]
    wqk_aug = np.ascontiguousarray(wqk.T).astype(bf)  # [768, 1536]
    wqkb = np.ascontiguousarray(wqk_bias.reshape(12, 128).T)  # [128, 12] f32

    wv = np.ascontiguousarray(qkv_w[2 * C:3 * C].T).astype(bf)  # [768, 768]

    rpb = rpb_table[idx]                              # [N, N, H] (q, k, h)
    biasT = np.ascontiguousarray(rpb.transpose(2, 1, 0))  # [H, k, q]
    expb_pad = np.zeros((H, NKT * 128, NP), f)
    expb_pad[:, :N, :N] = biasT
    expb_pad = np.exp(expb_pad)                       # pads exp(0) = 1
    expb = np.ascontiguousarray(
        expb_pad[:, :, :1024].reshape(H, NKT, 128, 1024)).astype(bf)
    # tail: q cols 1024:1028 for each k tile, packed [H, 128, 4*NKT]
    tail = expb_pad[:, :, 1024:1028].reshape(H, NKT, 128, 4)
    expb_tail = np.ascontiguousarray(
        tail.transpose(0, 2, 1, 3).reshape(H, 128, 4 * NKT)).astype(bf)

    wproj = np.ascontiguousarray(proj_w.T).astype(bf)  # [768, 768]
    # v_bias folded exactly into the projection bias:
    # out += v_bias @ proj_w.T == proj_w @ v_bias (per output channel)
    projb_eff = proj_b + proj_w @ v_bias
    projb = np.ascontiguousarray(projb_eff.reshape(C, 1)).astype(f)

    xT = np.zeros((B, C, NP), bf)
    for b in range(B):
        xT[b, :, :N] = x[b].T.astype(bf)
    return xT, wqk_aug, wqkb, wv, expb, expb_tail, wproj, projb


def _dedupe_ldweights(nc):
    """Remove InstLdweights that reload the identical stationary AP as the
    immediately preceding PE ldweights (only matmuls in between, which leave
    the loaded weights intact). The duplicate's dependants are remapped to
    the surviving ldweights; incoming deps are merged (identical anyway
    since the stationary AP is identical)."""
    import concourse.mybir as mybir
    PE = mybir.EngineType.PE
    n_del = 0
    for blk in nc.main_func.blocks:
        keep = []
        last_key = None
        last_ldw = None
        for inst in blk.instructions:
            tn = type(inst).__name__
            if getattr(inst, "engine", None) == PE:
                if tn == "InstLdweights":
                    key = str(inst.ins[0])
                    si = inst.sync_info
                    clean = si is None or (
                        len(si.on_wait) == 0 and len(si.on_update) == 0)
                    if key == last_key and clean and last_ldw is not None:
                        for dname in list(inst.descendants or []):
                            dep = nc.inst_map.get(dname)
                            if dep is not None:
                                try:
                                    dep.remap_dependency_names(
                                        {inst.name: last_ldw.name})
                                except Exception:
                                    pass
                        try:
                            last_ldw.merge_dependencies_from(inst)
                        except Exception:
                            pass
                        n_del += 1
                        continue
                    last_key = key
                    last_ldw = inst
                elif tn != "InstMatmult":
                    last_key = None
                    last_ldw = None
            keep.append(inst)
        blk.instructions[:] = keep
    return n_del


_BUILT = {}


def _build():
    if "nc" in _BUILT:
        return _BUILT["nc"]
    from contextlib import ExitStack

    import concourse.mybir as mybir
    import concourse.tile as tile
    from concourse import bacc

    nc = bacc.Bacc("TRN2", target_bir_lowering=False, debug=False,
                   num_devices=B)
    f32 = mybir.dt.float32
    bf16 = mybir.dt.bfloat16
    xT = nc.dram_tensor("xT", (C, NP), bf16, kind="ExternalInput").ap()
    wqk = nc.dram_tensor("wqk", (768, 1536), bf16, kind="ExternalInput").ap()
    wqkb = nc.dram_tensor("wqkb", (128, 12), f32, kind="ExternalInput").ap()
    wv = nc.dram_tensor("wv", (768, 768), bf16, kind="ExternalInput").ap()
    expb = nc.dram_tensor("expb", (H, NKT, 128, 1024), bf16,
                          kind="ExternalInput").ap()
    expb_tail = nc.dram_tensor("expb_tail", (H, 128, 4 * NKT), bf16,
                               kind="ExternalInput").ap()
    wproj = nc.dram_tensor("wproj", (768, 768), bf16,
                           kind="ExternalInput").ap()
    projb = nc.dram_tensor("projb", (768, 1), f32, kind="ExternalInput").ap()
    outT = nc.dram_tensor("outT", (768, N), f32, kind="ExternalOutput").ap()

    dbg = None
    if DEBUG:
        dbg = {
            "qk": nc.dram_tensor("dbg_qk", (12, 128, NP), bf16,
                                 kind="ExternalOutput").ap(),
            "v": nc.dram_tensor("dbg_v", (NKT, 128, H * (D + 1)), bf16,
                                kind="ExternalOutput").ap(),
            "pe": nc.dram_tensor("dbg_pe", (NKT, 128, 1024), bf16,
                                 kind="ExternalOutput").ap(),
            "ops": nc.dram_tensor("dbg_ops", (2, 65, 512), f32,
                                  kind="ExternalOutput").ap(),
            "rs": nc.dram_tensor("dbg_rs", (2, 1, 512), f32,
                                 kind="ExternalOutput").ap(),
            "bc": nc.dram_tensor("dbg_bc", (2, 64, 512), f32,
                                 kind="ExternalOutput").ap(),
            "oall": nc.dram_tensor("dbg_oall", (6, 128, NP), bf16,
                                   kind="ExternalOutput").ap(),
        }

    with tile.TileContext(nc) as tc:
        with ExitStack() as ctx:
            _emit(ctx, tc, xT, wqk, wqkb, wv, expb, expb_tail, wproj,
                  projb, outT, dbg=dbg)
    _dedupe_ldweights(nc)
    nc.compile()
    _BUILT["nc"] = nc
    return nc


def kernel(x, qkv_w, q_bias, v_bias, rpb_table, proj_w, proj_b,
           rel_pos_index):
    from concourse.bass_utils import run_bass_kernel_spmd

    xT, wqk, wqkb, wv, expb, expb_tail, wproj, projb = _host_prep(
        x, qkv_w, q_bias, v_bias, rpb_table, proj_w, proj_b, rel_pos_index)

    nc = _build()
    shared = {
        "wqk": wqk, "wqkb": wqkb, "wv": wv, "expb": expb,
        "expb_tail": expb_tail, "wproj": wproj, "projb": projb,
    }
    in_maps = [dict(shared, xT=np.ascontiguousarray(xT[b]))
               for b in range(B)]
    res = run_bass_kernel_spmd(nc, in_maps, core_ids=list(range(B)))
    out = np.stack([res.results[b]["outT"].T for b in range(B)], axis=0)
    return out.astype(np.float32)
